# revision 7
# baseline (speedup 1.0000x reference)
"""MoE routed decoder kernel for 8 Trainium2 NeuronCores (v2).

Sharding: every core runs the SAME program over three uniform regions, each a
(rows x W2-column-slice) block of one expert, with layer-2 kept transposed
(o^T = W2^T h) so the row count never pads up to 128:

  region A: 16 ocol-tiles (2048 cols), rows R_A  -> dense expert d1 (cores 0-3)
                                                    dense expert d2 (cores 4-7)
  region B:  8 ocol-tiles (1024 cols), rows R_B  -> largest dense d0 (all cores)
  region S: 16 ocol-tiles (2048 cols), rows R_S  -> sparse s0 (0-3), s1 (4-7)

This tiles all 5*8192 W2 columns exactly once across the device (20.97 MB per
core, the HBM floor), balances PE work exactly, and each core's layer 1 only
covers its own 3 experts' rows (~650) instead of all 1024+padding.

Layer 2 per ocol-tile: stationary = W2 128x128 chunk, moving = h rows, PSUM
gets o^T [128 ocols, R rows]. The complex-pair norm over each 256-col group is
a partition reduction done as a ones-vector matmul over the two tiles of the
group; the 1/norm row vector is broadcast back to 128 partitions with a rank-1
ones matmul, inverted on DVE, and multiplied into o^T. Outputs ship as bf16
o^T blocks; the host transposes, scatters rows back, and upcasts to f32.
"""

import os
import sys
import types

import numpy as np
import ml_dtypes

import concourse.bass as bass
import concourse.mybir as mybir
import concourse.tile as tile
from concourse import bacc
import concourse.bass_utils as bass_utils
from concourse.bass_utils import run_bass_kernel_spmd

B, D, H, O, E, P = 1024, 512, 2048, 8192, 5, 128
NCORES = 8
KC1 = D // P   # 4 contraction chunks in layer 1
KH = H // P    # 16 contraction chunks in layer 2
NTA, NTB, NTS = 16, 8, 16   # ocol-tiles per region (128 cols each)
RCHUNK = 512               # PSUM free-dim limit (f32)
BF16 = mybir.dt.bfloat16
F32 = mybir.dt.float32
AF = mybir.ActivationFunctionType

LAST_EXEC_NS = None
LAST_TRACE = None


def _install_ntff_hook():
    try:
        import trn_agent_boot.trn_boot as tb

        hook = tb._ntff_profile_via_ctypes("/opt/axon/libaxon_pjrt.so")
        mod = types.ModuleType("antenv.axon_hooks")
        mod.get_axon_ntff_profile_hook = lambda: hook
        import antenv

        antenv.axon_hooks = mod
        sys.modules["antenv.axon_hooks"] = mod
        bass_utils.upload_artifacts = lambda tmpdir: tmpdir
        return True
    except Exception:
        return False


def _route(x):
    c1 = x[:, -1].astype(np.int32) == 0
    c2 = x[:, -2].astype(np.int32) == 0
    c3 = x[:, -3].astype(np.int32) == 0
    r_if = np.where(c2, 0, np.where(c3, 3, 4))
    r_else = np.where(c2, 1, 2)
    return np.where(c1, r_if, r_else).astype(np.int64)


def _pad_idx(idx, R):
    n = len(idx)
    fill = int(idx[0]) if n else 0
    return np.concatenate([idx, np.full(R - n, fill, dtype=np.int64)])


def _pack_xT(x_bf, idx, R):
    """rows idx (padded to R) -> [P, KC1*R] bf16, xT[p, kc*R+r] = x[row_r, kc*128+p]."""
    xt = np.ascontiguousarray(x_bf[_pad_idx(idx, R)].T)  # (512, R)
    return np.ascontiguousarray(
        xt.reshape(KC1, P, R).transpose(1, 0, 2).reshape(P, KC1 * R)
    )


def _pack_w1(w1e_bf):
    """(512, 2048) -> [P, KC1*H]: [p, kc*H + hid] = W1[kc*128+p, hid]."""
    return np.ascontiguousarray(
        w1e_bf.reshape(KC1, P, H).transpose(1, 0, 2).reshape(P, KC1 * H)
    )


def _pack_w2(w2e_bf, c0, nt):
    """cols [c0, c0+nt*128) -> [P, nt*KH*128]: [p,(t*KH+kc)*128+m] = W2[kc*128+p, c0+t*128+m]."""
    sl = w2e_bf[:, c0:c0 + nt * P]  # (2048, nt*128)
    return np.ascontiguousarray(
        sl.reshape(KH, P, nt, P).transpose(1, 2, 0, 3).reshape(P, nt * KH * P)
    )


def _pack_w2g(w2e_bf, c0, C, tw):
    """cols [c0,c0+C) -> [P, (C//tw)*KH*tw]: [p,((t*KH)+kc)*tw+m] = W2[kc*128+p, c0+t*tw+m]."""
    sl = w2e_bf[:, c0:c0 + C]
    return np.ascontiguousarray(
        sl.reshape(KH, P, C // tw, tw).transpose(1, 2, 0, 3).reshape(P, -1)
    )


def _build(RA, RB, RS, b1_nz, b2_nz):
    CA, CB, CS = NTA * P, NTB * P, NTS * P
    NBA, NBB = -(-RA // P), -(-RB // P)
    nc = bacc.Bacc("TRN2", target_bir_lowering=False, debug=False,
                   num_devices=NCORES)
    regs_meta = [("a", RA, NTA), ("b", RB, NTB), ("s", RS, NTS)]
    dram = {}
    for tag, R, NT in regs_meta:
        dram[f"x{tag}"] = nc.dram_tensor(f"x{tag}", [P, KC1 * R], BF16,
                                         kind="ExternalInput").ap()
        dram[f"w1{tag}"] = nc.dram_tensor(f"w1{tag}", [P, KC1 * H], BF16,
                                          kind="ExternalInput").ap()
        dram[f"w2{tag}"] = nc.dram_tensor(f"w2{tag}", [P, NT * KH * P], BF16,
                                          kind="ExternalInput").ap()
    dram["outa"] = nc.dram_tensor("outa", [P, NBA * CA], BF16,
                                  kind="ExternalOutput").ap()
    dram["outb"] = nc.dram_tensor("outb", [P, NBB * CB], BF16,
                                  kind="ExternalOutput").ap()
    dram["outs"] = nc.dram_tensor("outs", [P, NTS * RS], BF16,
                                  kind="ExternalOutput").ap()
    if b1_nz:
        dram["b1"] = nc.dram_tensor("b1", [P, 3 * KH], F32,
                                    kind="ExternalInput").ap()
    if b2_nz:
        # base regions need per-column bias broadcast over rows; S region
        # needs per-ocol (partition) bias
        dram["b2ab"] = nc.dram_tensor("b2ab", [1, CA + CB], F32,
                                      kind="ExternalInput").ap()
        dram["b2s"] = nc.dram_tensor("b2s", [P, NTS], F32,
                                     kind="ExternalInput").ap()

    with tile.TileContext(nc) as tc:
        with (
            tc.tile_pool(name="singles", bufs=1) as singles,
            tc.tile_pool(name="w2ab", bufs=2) as w2ab,
            tc.tile_pool(name="w2sp", bufs=7) as w2sp,
            tc.tile_pool(name="sqp", bufs=4) as sqp,
            tc.tile_pool(name="nrmp", bufs=4) as nrmp,
            tc.tile_pool(name="rnp", bufs=4) as rnp,
            tc.tile_pool(name="ps1", bufs=2, space="PSUM") as ps1,
            tc.tile_pool(name="ps2", bufs=4, space="PSUM") as ps2,
            tc.tile_pool(name="psn", bufs=1, space="PSUM") as psn,
            tc.tile_pool(name="psb", bufs=1, space="PSUM") as psb,
        ):
            ones_col = singles.tile([P, 1], BF16, tag="ones_col")
            nc.vector.memset(ones_col[:], 1.0)
            ones_row = singles.tile([1, P], BF16, tag="ones_row")
            nc.vector.memset(ones_row[:], 1.0)

            b1_sb = None
            if b1_nz:
                b1_sb = singles.tile([P, 3 * KH], F32, tag="b1")
                nc.sync.dma_start(b1_sb[:], dram["b1"])
            b2ab_sb = b2s_sb = None
            if b2_nz:
                b2ab_sb = singles.tile([P, CA + CB], F32, tag="b2ab")
                bcast = bass.AP(tensor=dram["b2ab"].tensor,
                                offset=dram["b2ab"].offset,
                                ap=[[0, P], *dram["b2ab"].ap[1:]])
                nc.sync.dma_start(b2ab_sb[:], bcast)
                b2s_sb = singles.tile([P, NTS], F32, tag="b2s")
                nc.sync.dma_start(b2s_sb[:], dram["b2s"])

            xt_sb, w1_sb, h_sb = {}, {}, {}
            for tag, R, NT in regs_meta:
                t = singles.tile([P, KC1 * R], BF16, name=f"x_{tag}")
                nc.scalar.dma_start(t[:], dram[f"x{tag}"])
                xt_sb[tag] = t
                t = singles.tile([P, KC1 * H], BF16, name=f"w1_{tag}")
                nc.scalar.dma_start(t[:], dram[f"w1{tag}"])
                w1_sb[tag] = t
                h_sb[tag] = singles.tile([P, KH * R], BF16, name=f"h_{tag}")

            # ---- W2 chunk stream: all DMAs issued upfront on the sync
            # queue in an order that prefetches S during the L1/A/B phases;
            # pool-slot reuse (AB window=2, S window=7) is the flow control.
            w2_tiles = {}
            issue_order = [("a", 0), ("s", 0), ("s", 1), ("a", 1), ("s", 2),
                           ("s", 3), ("s", 4), ("s", 5), ("s", 6), ("a", 2),
                           ("a", 3), ("b", 0), ("b", 1), ("s", 7)]
            for rt, cc in issue_order:
                if rt == "s":
                    t = w2sp.tile([P, 2 * KH * P], BF16, tag="w2s",
                                  name=f"w2s_{cc}")
                    nc.sync.dma_start(
                        t[:], dram["w2s"][:, cc * 2 * KH * P:(cc + 1) * 2 * KH * P])
                else:
                    t = w2ab.tile([P, KH * 512], BF16, tag="w2ab",
                                  name=f"w2{rt}_{cc}")
                    nc.sync.dma_start(
                        t[:], dram[f"w2{rt}"][:, cc * KH * 512:(cc + 1) * KH * 512])
                w2_tiles[(rt, cc)] = t

            # ---- layer 1: h^T = relu(W1^T x^T) per region
            for ri, (tag, R, NT) in enumerate(regs_meta):
                xt, w1, h = xt_sb[tag], w1_sb[tag], h_sb[tag]
                for hm in range(KH):
                    for r0 in range(0, R, RCHUNK):
                        rn = min(RCHUNK, R - r0)
                        ph = ps1.tile([P, RCHUNK], F32, tag="ps1")
                        for kc in range(KC1):
                            nc.tensor.matmul(
                                ph[:, :rn],
                                w1[:, kc * H + hm * P:kc * H + (hm + 1) * P],
                                xt[:, kc * R + r0:kc * R + r0 + rn],
                                start=(kc == 0), stop=(kc == KC1 - 1),
                            )
                        bias = b1_sb[:, ri * KH + hm:ri * KH + hm + 1] \
                            if b1_nz else 0.0
                        nc.scalar.activation(
                            h[:, hm * R + r0:hm * R + r0 + rn],
                            ph[:, :rn], AF.Relu, bias=bias,
                        )

            # ---- layer 2, regions A/B: row-block orientation.
            # stationary = h block [128k, <=128 rows], moving = W2 512-col
            # chunks (LDWEIGHTS fully hidden under the 512 stream); norm is a
            # free-axis square-accumulate exactly like the classic epilogue.
            for tag, R, NT, C, boff in (("a", RA, NTA, CA, 0),
                                        ("b", RB, NTB, CB, CA)):
                h = h_sb[tag]
                NB = -(-R // P)
                out_sb = singles.tile([P, NB * C], BF16, name=f"o_{tag}")
                for cc in range(C // 512):
                    w2c = w2_tiles[(tag, cc)]
                    for b in range(NB):
                        b0 = b * P
                        bm = min(P, R - b0)
                        ps = ps2.tile([P, 512], F32, tag="ps2")
                        for kc in range(KH):
                            nc.tensor.matmul(
                                ps[:bm, :],
                                h[:, kc * R + b0:kc * R + b0 + bm],
                                w2c[:, kc * 512:(kc + 1) * 512],
                                start=(kc == 0), stop=(kc == KH - 1),
                            )
                        if b2_nz:
                            nc.vector.tensor_add(
                                ps[:bm, :], ps[:bm, :],
                                b2ab_sb[:bm, boff + cc * 512:boff + cc * 512 + 512],
                            )
                        nrm = nrmp.tile([P, 2], F32, tag="nrm")
                        for j in range(2):
                            sqd = sqp.tile([P, 256], BF16, tag="sqd")
                            nc.scalar.activation(
                                sqd[:bm, :], ps[:bm, j * 256:(j + 1) * 256],
                                AF.Square, accum_out=nrm[:bm, j:j + 1],
                            )
                        nc.scalar.sqrt(nrm[:bm, :], nrm[:bm, :])
                        rn = rnp.tile([P, 2], F32, tag="rn")
                        nc.vector.reciprocal(rn[:bm, :], nrm[:bm, :])
                        for j in range(2):
                            nc.vector.tensor_scalar_mul(
                                out_sb[:bm, b * C + cc * 512 + j * 256:
                                       b * C + cc * 512 + (j + 1) * 256],
                                ps[:bm, j * 256:(j + 1) * 256],
                                rn[:bm, j:j + 1],
                            )
                nc.scalar.dma_start(dram[f"out{tag}"], out_sb[:])

            # ---- layer 2, region S: transposed orientation (rows stream).
            h = h_sb["s"]
            R = RS
            out_sb = singles.tile([P, NTS * R], BF16, name="o_s")
            for ch in range(NTS // 2):
                w2c = w2_tiles[("s", ch)]
                for r0 in range(0, R, RCHUNK):
                    rn_ = min(RCHUNK, R - r0)
                    pss = []
                    for ti in range(2):
                        ps = ps2.tile([P, RCHUNK], F32, tag="ps2")
                        for kc in range(KH):
                            nc.tensor.matmul(
                                ps[:, :rn_],
                                w2c[:, (ti * KH + kc) * P:(ti * KH + kc + 1) * P],
                                h[:, kc * R + r0:kc * R + r0 + rn_],
                                start=(kc == 0), stop=(kc == KH - 1),
                            )
                        if b2_nz:
                            nc.vector.tensor_scalar_add(
                                ps[:, :rn_], ps[:, :rn_],
                                b2s_sb[:, 2 * ch + ti:2 * ch + ti + 1],
                            )
                        pss.append(ps)
                    nrm = psn.tile([1, RCHUNK], F32, tag="nrms")
                    obs = []
                    for ps in pss:
                        ob = sqp.tile([P, RCHUNK], BF16, tag="ob")
                        nc.scalar.copy(ob[:, :rn_], ps[:, :rn_])
                        obs.append(ob)
                    sqs = []
                    for ob in obs:
                        sq = sqp.tile([P, RCHUNK], BF16, tag="sq")
                        nc.vector.tensor_mul(sq[:, :rn_], ob[:, :rn_],
                                             ob[:, :rn_])
                        sqs.append(sq)
                    for ti, sq in enumerate(sqs):
                        nc.tensor.matmul(
                            nrm[:, :rn_], ones_col[:, 0:1], sq[:, :rn_],
                            start=(ti == 0), stop=(ti == 1),
                        )
                    # 1/sqrt: DVE reciprocal on the [1,R] vector (cheap),
                    # then ACT sqrt straight to bf16 for the broadcast matmul
                    ni = rnp.tile([1, RCHUNK], F32, tag="ni")
                    nc.vector.reciprocal(ni[:, :rn_], nrm[:, :rn_])
                    sn = rnp.tile([1, RCHUNK], BF16, tag="sn")
                    nc.scalar.sqrt(sn[:, :rn_], ni[:, :rn_])
                    rnb = psb.tile([P, RCHUNK], F32, tag="rnb")
                    nc.tensor.matmul(rnb[:, :rn_], ones_row[0:1, :],
                                     sn[0:1, :rn_], start=True, stop=True)
                    for ti, ob in enumerate(obs):
                        t = ch * 2 + ti
                        nc.vector.tensor_mul(
                            out_sb[:, t * R + r0:t * R + r0 + rn_],
                            ob[:, :rn_], rnb[:, :rn_],
                        )
            nc.scalar.dma_start(dram["outs"], out_sb[:])

    nc.compile()
    return nc


def _roundup8(n):
    return max(8, -(-n // 8) * 8)


def kernel(x, W1, b1, W2, b2):
    x = np.asarray(x, dtype=np.float32)
    W1 = np.asarray(W1, dtype=np.float32)
    b1 = np.asarray(b1, dtype=np.float32)
    W2 = np.asarray(W2, dtype=np.float32)
    b2 = np.asarray(b2, dtype=np.float32)

    route = _route(x)
    idx = [np.nonzero(route == e)[0] for e in range(E)]
    counts = np.array([len(i) for i in idx])
    order = np.argsort(-counts, kind="stable")
    d0, d1, d2, s0, s1 = [int(e) for e in order]

    RA = _roundup8(max(counts[d1], counts[d2]))
    RB = _roundup8(counts[d0])
    RS = _roundup8(max(counts[s0], counts[s1]))

    b1_nz = bool(np.any(b1))
    b2_nz = bool(np.any(b2))

    x_bf = x.astype(ml_dtypes.bfloat16)
    w1_bf = W1.astype(ml_dtypes.bfloat16)
    w2_bf = W2.astype(ml_dtypes.bfloat16)

    # region -> (expert per core-group, cols per core, rows)
    xA = {e: _pack_xT(x_bf, idx[e], RA) for e in (d1, d2)}
    xB = _pack_xT(x_bf, idx[d0], RB)
    xS = {e: _pack_xT(x_bf, idx[e], RS) for e in (s0, s1)}
    w1p = {e: _pack_w1(w1_bf[e]) for e in range(E)}

    nc = _build(RA, RB, RS, b1_nz, b2_nz)

    in_maps = []
    for c in range(NCORES):
        g = 0 if c < 4 else 1
        eA = (d1, d2)[g]
        eS = (s0, s1)[g]
        colA = (c % 4) * (NTA * P)
        colB = c * (NTB * P)
        colS = (c % 4) * (NTS * P)
        m = {
            "xa": xA[eA], "xb": xB, "xs": xS[eS],
            "w1a": w1p[eA], "w1b": w1p[d0], "w1s": w1p[eS],
            "w2a": _pack_w2g(w2_bf[eA], colA, NTA * P, 512),
            "w2b": _pack_w2g(w2_bf[d0], colB, NTB * P, 512),
            "w2s": _pack_w2g(w2_bf[eS], colS, NTS * P, P),
        }
        if b1_nz:
            m["b1"] = np.ascontiguousarray(np.concatenate(
                [b1[e].reshape(KH, P).T for e in (eA, d0, eS)], axis=1))
        if b2_nz:
            m["b2ab"] = np.ascontiguousarray(np.concatenate(
                [b2[eA][colA:colA + NTA * P],
                 b2[d0][colB:colB + NTB * P]])[None, :])
            m["b2s"] = np.ascontiguousarray(
                b2[eS][colS:colS + NTS * P].reshape(NTS, P).T)
        in_maps.append(m)

    trace = os.environ.get("BASSMOE_TRACE", "") == "1"
    if trace:
        trace = _install_ntff_hook()

    res = run_bass_kernel_spmd(
        nc, in_maps, core_ids=list(range(NCORES)), trace=trace,
        tmpdir=os.environ.get("BASSMOE_TRACE_DIR") or None,
    )
    global LAST_EXEC_NS, LAST_TRACE
    LAST_EXEC_NS = res.exec_time_ns
    LAST_TRACE = res.instructions_and_trace[1] if res.instructions_and_trace else None

    out = np.empty((B, O), dtype=np.float32)
    for c in range(NCORES):
        g = 0 if c < 4 else 1
        # base-orientation regions: out[p, b*C + c] = row b*128+p, col c0+c
        for tag, e, col0, C in (
            ("outa", (d1, d2)[g], (c % 4) * NTA * P, NTA * P),
            ("outb", d0, c * NTB * P, NTB * P),
        ):
            n = len(idx[e])
            if n == 0:
                continue
            NB = -(-([RA, RB][tag == "outb"]) // P)
            blk = np.asarray(res.results[c][tag]).reshape(P, NB, C)
            rows = np.asarray(idx[e])
            for b in range(NB):
                bm = min(P, n - b * P)
                if bm <= 0:
                    break
                out[rows[b * P:b * P + bm], col0:col0 + C] = (
                    blk[:bm, b, :].astype(np.float32))
        # transposed region S: out[p, t*R + r] = row r, col c0 + t*128 + p
        e = (s0, s1)[g]
        n = len(idx[e])
        if n:
            col0 = (c % 4) * NTS * P
            blk = np.asarray(res.results[c]["outs"]).reshape(P, NTS, RS)[:, :, :n]
            out[np.asarray(idx[e]), col0:col0 + NTS * P] = (
                blk.transpose(2, 1, 0).reshape(n, NTS * P).astype(np.float32)
            )
    return out.reshape(B, 32, 256)


# revision 8
# speedup vs baseline: 1.1594x; 1.1594x over previous
"""MoE routed decoder kernel for 8 Trainium2 NeuronCores (v2).

Sharding: every core runs the SAME program over three uniform regions, each a
(rows x W2-column-slice) block of one expert, with layer-2 kept transposed
(o^T = W2^T h) so the row count never pads up to 128:

  region A: 16 ocol-tiles (2048 cols), rows R_A  -> dense expert d1 (cores 0-3)
                                                    dense expert d2 (cores 4-7)
  region B:  8 ocol-tiles (1024 cols), rows R_B  -> largest dense d0 (all cores)
  region S: 16 ocol-tiles (2048 cols), rows R_S  -> sparse s0 (0-3), s1 (4-7)

This tiles all 5*8192 W2 columns exactly once across the device (20.97 MB per
core, the HBM floor), balances PE work exactly, and each core's layer 1 only
covers its own 3 experts' rows (~650) instead of all 1024+padding.

Layer 2 per ocol-tile: stationary = W2 128x128 chunk, moving = h rows, PSUM
gets o^T [128 ocols, R rows]. The complex-pair norm over each 256-col group is
a partition reduction done as a ones-vector matmul over the two tiles of the
group; the 1/norm row vector is broadcast back to 128 partitions with a rank-1
ones matmul, inverted on DVE, and multiplied into o^T. Outputs ship as bf16
o^T blocks; the host transposes, scatters rows back, and upcasts to f32.
"""

import os
import sys
import types

import numpy as np
import ml_dtypes

import concourse.bass as bass
import concourse.mybir as mybir
import concourse.tile as tile
from concourse import bacc
import concourse.bass_utils as bass_utils
from concourse.bass_utils import run_bass_kernel_spmd

B, D, H, O, E, P = 1024, 512, 2048, 8192, 5, 128
NCORES = 8
KC1 = D // P   # 4 contraction chunks in layer 1
KH = H // P    # 16 contraction chunks in layer 2
NTA, NTB, NTS = 16, 8, 16   # ocol-tiles per region (128 cols each)
RCHUNK = 512               # PSUM free-dim limit (f32)
BF16 = mybir.dt.bfloat16
F32 = mybir.dt.float32
AF = mybir.ActivationFunctionType

LAST_EXEC_NS = None
LAST_TRACE = None


def _install_ntff_hook():
    try:
        import trn_agent_boot.trn_boot as tb

        hook = tb._ntff_profile_via_ctypes("/opt/axon/libaxon_pjrt.so")
        mod = types.ModuleType("antenv.axon_hooks")
        mod.get_axon_ntff_profile_hook = lambda: hook
        import antenv

        antenv.axon_hooks = mod
        sys.modules["antenv.axon_hooks"] = mod
        bass_utils.upload_artifacts = lambda tmpdir: tmpdir
        return True
    except Exception:
        return False


def _route(x):
    c1 = x[:, -1].astype(np.int32) == 0
    c2 = x[:, -2].astype(np.int32) == 0
    c3 = x[:, -3].astype(np.int32) == 0
    r_if = np.where(c2, 0, np.where(c3, 3, 4))
    r_else = np.where(c2, 1, 2)
    return np.where(c1, r_if, r_else).astype(np.int64)


def _pad_idx(idx, R):
    n = len(idx)
    fill = int(idx[0]) if n else 0
    return np.concatenate([idx, np.full(R - n, fill, dtype=np.int64)])


def _pack_xT(x_bf, idx, R):
    """rows idx (padded to R) -> [P, KC1*R] bf16, xT[p, kc*R+r] = x[row_r, kc*128+p]."""
    xt = np.ascontiguousarray(x_bf[_pad_idx(idx, R)].T)  # (512, R)
    return np.ascontiguousarray(
        xt.reshape(KC1, P, R).transpose(1, 0, 2).reshape(P, KC1 * R)
    )


def _pack_w1(w1e_bf):
    """(512, 2048) -> [P, KC1*H]: [p, kc*H + hid] = W1[kc*128+p, hid]."""
    return np.ascontiguousarray(
        w1e_bf.reshape(KC1, P, H).transpose(1, 0, 2).reshape(P, KC1 * H)
    )


def _pack_w2(w2e_bf, c0, nt):
    """cols [c0, c0+nt*128) -> [P, nt*KH*128]: [p,(t*KH+kc)*128+m] = W2[kc*128+p, c0+t*128+m]."""
    sl = w2e_bf[:, c0:c0 + nt * P]  # (2048, nt*128)
    return np.ascontiguousarray(
        sl.reshape(KH, P, nt, P).transpose(1, 2, 0, 3).reshape(P, nt * KH * P)
    )


def _pack_w2g(w2e_bf, c0, C, tw):
    """cols [c0,c0+C) -> [P, (C//tw)*KH*tw]: [p,((t*KH)+kc)*tw+m] = W2[kc*128+p, c0+t*tw+m]."""
    sl = w2e_bf[:, c0:c0 + C]
    return np.ascontiguousarray(
        sl.reshape(KH, P, C // tw, tw).transpose(1, 2, 0, 3).reshape(P, -1)
    )


def _build(RA, RB, RS, b1_nz, b2_nz):
    CA, CB, CS = NTA * P, NTB * P, NTS * P
    nc = bacc.Bacc("TRN2", target_bir_lowering=False, debug=False,
                   num_devices=NCORES)
    regs_meta = [("a", RA, NTA), ("b", RB, NTB), ("s", RS, NTS)]
    dram = {}
    for tag, R, NT in regs_meta:
        NB = -(-R // P)
        dram[f"x{tag}"] = nc.dram_tensor(f"x{tag}", [P, KC1 * R], BF16,
                                         kind="ExternalInput").ap()
        dram[f"w1{tag}"] = nc.dram_tensor(f"w1{tag}", [P, KC1 * H], BF16,
                                          kind="ExternalInput").ap()
        dram[f"w2{tag}"] = nc.dram_tensor(f"w2{tag}", [P, NT * KH * P], BF16,
                                          kind="ExternalInput").ap()
        dram[f"out{tag}"] = nc.dram_tensor(f"out{tag}", [P, NB * NT * P], BF16,
                                           kind="ExternalOutput").ap()
    if b1_nz:
        dram["b1"] = nc.dram_tensor("b1", [P, 3 * KH], F32,
                                    kind="ExternalInput").ap()
    if b2_nz:
        dram["b2"] = nc.dram_tensor("b2", [1, CA + CB + CS], F32,
                                    kind="ExternalInput").ap()

    with tile.TileContext(nc) as tc:
        with (
            tc.tile_pool(name="singles", bufs=1) as singles,
            tc.tile_pool(name="w2p", bufs=4) as w2p,
            tc.tile_pool(name="sqp", bufs=4) as sqp,
            tc.tile_pool(name="cmb", bufs=4) as cmb,
            tc.tile_pool(name="nrmp", bufs=4) as nrmp,
            tc.tile_pool(name="rnp", bufs=4) as rnp,
            tc.tile_pool(name="ps1", bufs=2, space="PSUM") as ps1,
            tc.tile_pool(name="ps2", bufs=6, space="PSUM") as ps2,
        ):
            b1_sb = None
            if b1_nz:
                b1_sb = singles.tile([P, 3 * KH], F32, tag="b1")
                nc.sync.dma_start(b1_sb[:], dram["b1"])
            b2_sb = None
            if b2_nz:
                b2_sb = singles.tile([P, CA + CB + CS], F32, tag="b2")
                bcast = bass.AP(tensor=dram["b2"].tensor,
                                offset=dram["b2"].offset,
                                ap=[[0, P], *dram["b2"].ap[1:]])
                nc.sync.dma_start(b2_sb[:], bcast)

            # ---- input DMA order IS the bandwidth priority (single HWDGE
            # queue): layer-1 inputs for A first, first W2 chunks, then the
            # rest; the w2 pool window (4 x 2MB) is the flow control and its
            # allocation order matches consumption order exactly.
            xt_sb, w1_sb, h_sb = {}, {}, {}

            def load_xw1(tag, R):
                t = singles.tile([P, KC1 * R], BF16, name=f"x_{tag}")
                nc.sync.dma_start(t[:], dram[f"x{tag}"])
                xt_sb[tag] = t
                t = singles.tile([P, KC1 * H], BF16, name=f"w1_{tag}")
                nc.sync.dma_start(t[:], dram[f"w1{tag}"])
                w1_sb[tag] = t

            w2_tiles = {}

            def load_w2(tag, cc):
                t = w2p.tile([P, KH * 512], BF16, tag="w2",
                             name=f"w2{tag}_{cc}")
                nc.sync.dma_start(
                    t[:], dram[f"w2{tag}"][:, cc * KH * 512:(cc + 1) * KH * 512])
                w2_tiles[(tag, cc)] = t

            load_xw1("a", RA)
            load_w2("a", 0)
            load_w2("a", 1)
            load_xw1("b", RB)
            load_xw1("s", RS)
            for key in [("a", 2), ("a", 3), ("b", 0), ("b", 1),
                        ("s", 0), ("s", 1), ("s", 2), ("s", 3)]:
                load_w2(*key)

            for tag, R, NT in regs_meta:
                h_sb[tag] = singles.tile([P, KH * R], BF16, name=f"h_{tag}")

            # ---- layer 1: h^T = relu(W1^T x^T) per region
            for ri, (tag, R, NT) in enumerate(regs_meta):
                xt, w1, h = xt_sb[tag], w1_sb[tag], h_sb[tag]
                for hm in range(KH):
                    for r0 in range(0, R, RCHUNK):
                        rn = min(RCHUNK, R - r0)
                        ph = ps1.tile([P, RCHUNK], F32, tag="ps1")
                        for kc in range(KC1):
                            nc.tensor.matmul(
                                ph[:, :rn],
                                w1[:, kc * H + hm * P:kc * H + (hm + 1) * P],
                                xt[:, kc * R + r0:kc * R + r0 + rn],
                                start=(kc == 0), stop=(kc == KC1 - 1),
                            )
                        bias = b1_sb[:, ri * KH + hm:ri * KH + hm + 1] \
                            if b1_nz else 0.0
                        nc.scalar.activation(
                            h[:, hm * R + r0:hm * R + r0 + rn],
                            ph[:, :rn], AF.Relu, bias=bias,
                        )

            # ---- layer 2: row-block orientation everywhere. Full 128-row
            # blocks stream 512 W2 columns per matmul (LDWEIGHTS hidden); a
            # <=32-row remainder block packs its 16 k-chunks 4-per-col-group
            # with tile_position so the 4 partial matmuls run concurrently,
            # then DVE/ACT combine the partials.
            def epilogue(src_ap, bm, out_sb, off, boff, cc):
                # src_ap: [bm, 512] (PSUM or SBUF) holding 2 norm groups
                nrm = nrmp.tile([P, 2], F32, tag="nrm")
                for j in range(2):
                    sqd = sqp.tile([P, 256], BF16, tag="sqd")
                    nc.scalar.activation(
                        sqd[:bm, :], src_ap[:, j * 256:(j + 1) * 256],
                        AF.Square, accum_out=nrm[:bm, j:j + 1])
                nc.scalar.sqrt(nrm[:bm, :], nrm[:bm, :])
                rn = rnp.tile([P, 2], F32, tag="rn")
                nc.vector.reciprocal(rn[:bm, :], nrm[:bm, :])
                for j in range(2):
                    nc.vector.tensor_scalar_mul(
                        out_sb[:bm, off + j * 256:off + (j + 1) * 256],
                        src_ap[:, j * 256:(j + 1) * 256], rn[:bm, j:j + 1])

            for ri, (tag, R, NT) in enumerate(regs_meta):
                h = h_sb[tag]
                C = NT * P
                NB = -(-R // P)
                boff = [0, CA, CA + CB][ri]
                out_sb = singles.tile([P, NB * C], BF16, name=f"o_{tag}")
                for cc in range(C // 512):
                    w2c = w2_tiles[(tag, cc)]
                    for b in range(NB):
                        b0 = b * P
                        bm = min(P, R - b0)
                        ps = ps2.tile([P, 512], F32, tag="ps2")
                        off = b * C + cc * 512
                        if bm > 32:
                            for kc in range(KH):
                                nc.tensor.matmul(
                                    ps[:bm, :],
                                    h[:, kc * R + b0:kc * R + b0 + bm],
                                    w2c[:, kc * 512:(kc + 1) * 512],
                                    start=(kc == 0), stop=(kc == KH - 1),
                                )
                            if b2_nz:
                                nc.vector.tensor_add(
                                    ps[:bm, :], ps[:bm, :],
                                    b2_sb[:bm, boff + cc * 512:boff + cc * 512 + 512])
                            epilogue(ps[:bm, :], bm, out_sb, off, boff, cc)
                        else:
                            # remainder block: 4 col-groups x 4 k-chunks
                            for k4 in range(4):
                                for g in range(4):
                                    kc = g * 4 + k4
                                    nc.tensor.matmul(
                                        ps[32 * g:32 * g + bm, :],
                                        h[:, kc * R + b0:kc * R + b0 + bm],
                                        w2c[:, kc * 512:(kc + 1) * 512],
                                        start=(k4 == 0), stop=(k4 == 3),
                                        tile_position=(0, 32 * g),
                                    )
                            t0 = cmb.tile([P, 512], F32, tag="t0")
                            t1 = cmb.tile([P, 512], F32, tag="t1")
                            nc.scalar.copy(t0[:bm, :], ps[32:32 + bm, :])
                            nc.vector.tensor_add(t0[:bm, :], ps[0:bm, :],
                                                 t0[:bm, :])
                            nc.scalar.copy(t1[:bm, :], ps[96:96 + bm, :])
                            nc.vector.tensor_add(t1[:bm, :], ps[64:64 + bm, :],
                                                 t1[:bm, :])
                            nc.vector.tensor_add(t0[:bm, :], t0[:bm, :],
                                                 t1[:bm, :])
                            if b2_nz:
                                nc.vector.tensor_add(
                                    t0[:bm, :], t0[:bm, :],
                                    b2_sb[:bm, boff + cc * 512:boff + cc * 512 + 512])
                            epilogue(t0[:bm, :], bm, out_sb, off, boff, cc)
                nc.scalar.dma_start(dram[f"out{tag}"], out_sb[:])

    nc.compile()
    return nc


def _roundup8(n):
    return max(8, -(-n // 8) * 8)


def kernel(x, W1, b1, W2, b2):
    x = np.asarray(x, dtype=np.float32)
    W1 = np.asarray(W1, dtype=np.float32)
    b1 = np.asarray(b1, dtype=np.float32)
    W2 = np.asarray(W2, dtype=np.float32)
    b2 = np.asarray(b2, dtype=np.float32)

    route = _route(x)
    idx = [np.nonzero(route == e)[0] for e in range(E)]
    counts = np.array([len(i) for i in idx])
    order = np.argsort(-counts, kind="stable")
    d0, d1, d2, s0, s1 = [int(e) for e in order]

    RA = _roundup8(max(counts[d1], counts[d2]))
    RB = _roundup8(counts[d0])
    RS = _roundup8(max(counts[s0], counts[s1]))

    b1_nz = bool(np.any(b1))
    b2_nz = bool(np.any(b2))

    x_bf = x.astype(ml_dtypes.bfloat16)
    w1_bf = W1.astype(ml_dtypes.bfloat16)
    w2_bf = W2.astype(ml_dtypes.bfloat16)

    # region -> (expert per core-group, cols per core, rows)
    xA = {e: _pack_xT(x_bf, idx[e], RA) for e in (d1, d2)}
    xB = _pack_xT(x_bf, idx[d0], RB)
    xS = {e: _pack_xT(x_bf, idx[e], RS) for e in (s0, s1)}
    w1p = {e: _pack_w1(w1_bf[e]) for e in range(E)}

    nc = _build(RA, RB, RS, b1_nz, b2_nz)

    in_maps = []
    for c in range(NCORES):
        g = 0 if c < 4 else 1
        eA = (d1, d2)[g]
        eS = (s0, s1)[g]
        colA = (c % 4) * (NTA * P)
        colB = c * (NTB * P)
        colS = (c % 4) * (NTS * P)
        m = {
            "xa": xA[eA], "xb": xB, "xs": xS[eS],
            "w1a": w1p[eA], "w1b": w1p[d0], "w1s": w1p[eS],
            "w2a": _pack_w2g(w2_bf[eA], colA, NTA * P, 512),
            "w2b": _pack_w2g(w2_bf[d0], colB, NTB * P, 512),
            "w2s": _pack_w2g(w2_bf[eS], colS, NTS * P, 512),
        }
        if b1_nz:
            m["b1"] = np.ascontiguousarray(np.concatenate(
                [b1[e].reshape(KH, P).T for e in (eA, d0, eS)], axis=1))
        if b2_nz:
            m["b2"] = np.ascontiguousarray(np.concatenate(
                [b2[eA][colA:colA + NTA * P],
                 b2[d0][colB:colB + NTB * P],
                 b2[eS][colS:colS + NTS * P]])[None, :])
        in_maps.append(m)

    trace = os.environ.get("BASSMOE_TRACE", "") == "1"
    if trace:
        trace = _install_ntff_hook()

    res = run_bass_kernel_spmd(
        nc, in_maps, core_ids=list(range(NCORES)), trace=trace,
        tmpdir=os.environ.get("BASSMOE_TRACE_DIR") or None,
    )
    global LAST_EXEC_NS, LAST_TRACE
    LAST_EXEC_NS = res.exec_time_ns
    LAST_TRACE = res.instructions_and_trace[1] if res.instructions_and_trace else None

    out = np.empty((B, O), dtype=np.float32)
    for c in range(NCORES):
        g = 0 if c < 4 else 1
        # base-orientation regions: out[p, b*C + c] = row b*128+p, col c0+c
        for tag, e, col0, C, R in (
            ("outa", (d1, d2)[g], (c % 4) * NTA * P, NTA * P, RA),
            ("outb", d0, c * NTB * P, NTB * P, RB),
            ("outs", (s0, s1)[g], (c % 4) * NTS * P, NTS * P, RS),
        ):
            n = len(idx[e])
            if n == 0:
                continue
            NB = -(-R // P)
            blk = np.asarray(res.results[c][tag]).reshape(P, NB, C)
            rows = np.asarray(idx[e])
            for b in range(NB):
                bm = min(P, n - b * P)
                if bm <= 0:
                    break
                out[rows[b * P:b * P + bm], col0:col0 + C] = (
                    blk[:bm, b, :].astype(np.float32))
    return out.reshape(B, 32, 256)
